# revision 8
# baseline (speedup 1.0000x reference)
"""Trainium2 Bass kernel for nn_DeepConv1d (self-contained).

Math (per batch b):
  xr   = linear-interp(deep, 1024 -> 4096)           # commutes with 1x1 conv
  y    = conv_w @ xr + conv_b                        # == interp(conv_w @ deep + conv_b)
  xs   = GAMA*(y-mean)/(var_unbiased+EPS)            # per-channel over n
  loss_k[c,l] = sech^2(xs_pad[c,l+k]-xs_pad[c,l+3])  # k=0..6, reflect pad 3
  S    = sum_k loss_k ;  W_k = (loss_k/S)*x_pad[:,l+k]
  out[o,l] = sum_{c,k} fc_w[o, 7c+k] * W_k[c,l]

On-chip identities:
  - interp(conv(.)) == conv(interp(.)); interp via first differences D.
  - sech^2(d) = 4*sigmoid(2d)*sigmoid(-2d); the normalization scale
    f = GAMA/(var+EPS) folds into the sigmoid's per-partition scale
    (the mean cancels inside differences).
  - loss_k arrays are shifted views of 3 gap arrays lv_g (g = |k-3|):
      k<3: loss_k[l] = lv_g[l+k] (g=3-k);  k>3: loss_k[l] = lv_g[l+3] (g=k-3).

Layout: 2 batches per core packed on 128 partitions (64 channels each).
GEMM: per 512-wide l-chunk, 7 accumulating bf16 matmuls (K=64) into PSUM,
DMA'd straight to DRAM.
"""
import contextlib

import numpy as np
import ml_dtypes

import concourse.bass as bass
import concourse.bacc as bacc_mod
import concourse.mybir as mybir
import concourse.tile as tile
from concourse.bass_utils import run_bass_kernel_spmd

bf16 = ml_dtypes.bfloat16
AF = mybir.ActivationFunctionType
ALU = mybir.AluOpType

KS = 7
PAD = 3
GAMA = 0.5
EPS = 1e-9
N = 4096
ND = 1024
NP = N + 2 * PAD       # 4102
L3 = N + PAD           # 4099: lv array length
NCORES = 8

F32 = mybir.dt.float32
BF = mybir.dt.bfloat16

# Fraction of each elementwise op's free dim done on DVE (rest on GPSIMD).
SPLIT = {
    "dy": 0.70,       # gap diffs
    "madd": 0.70,     # S partial adds
    "prod": 0.75,     # product ops; MUST be a multiple of 512/H so each
                      # 512-wide GEMM chunk of W has exactly one producer
    "interp": 1.0,    # handled explicitly below
}


def _even(v):
    return int(v) & ~1


def _tt(eng, op, out, a, b):
    if op == "add":
        eng.tensor_add(out=out, in0=a, in1=b)
    elif op == "sub":
        eng.tensor_sub(out=out, in0=a, in1=b)
    elif op == "mul":
        eng.tensor_mul(out=out, in0=a, in1=b)
    else:
        raise ValueError(op)


def split_tt(nc, key, op, out, a, b, width):
    frac = SPLIT[key]
    c = _even(width * frac)
    if key == "prod":
        c = (c // 512) * 512
    c = min(c, width)
    if c > 0:
        _tt(nc.vector, op, out[:, 0:c], a[:, 0:c], b[:, 0:c])
    if c < width:
        _tt(nc.gpsimd, op, out[:, c:width], a[:, c:width], b[:, c:width])


def kernel_body(tc, xp_d, cwdp_d, cb_d, fck_d, out_d):
    nc = tc.nc
    ctx = contextlib.ExitStack()
    with ctx:
        io = ctx.enter_context(tc.tile_pool(name="io", bufs=1))
        mid = ctx.enter_context(tc.tile_pool(name="mid", bufs=1))
        loss = ctx.enter_context(tc.tile_pool(name="loss", bufs=1))
        wp = ctx.enter_context(tc.tile_pool(name="wp", bufs=1))
        pp = ctx.enter_context(tc.tile_pool(name="pp", bufs=1, space="PSUM"))
        ppa = ctx.enter_context(tc.tile_pool(name="ppa", bufs=2, space="PSUM"))
        stp = ctx.enter_context(tc.tile_pool(name="stp", bufs=3))

        # ---------------- input DMAs ----------------
        xp = io.tile([128, NP], BF, tag="xp")          # x reflect-padded
        xs1 = io.tile([128, NP - 1], BF, tag="xs1")    # same, shifted 1 elem
        nc.sync.dma_start(out=xp, in_=xp_d[:, :])
        nc.sync.dma_start(out=xs1, in_=xp_d[:, 1:NP])
        cwdp = io.tile([32, 128 + ND], F32, tag="cwdp")
        nc.sync.dma_start(out=cwdp, in_=cwdp_d[:, :])
        cw = cwdp[:, 0:128]
        dp = cwdp[:, 128:128 + ND]
        cb = io.tile([128, 1], F32, tag="cb")
        nc.sync.dma_start(out=cb, in_=cb_d[:, :])
        fck = io.tile([128, KS, 128], BF, tag="fck")
        nc.sync.dma_start(out=fck, in_=fck_d[:, :, :])

        # ---------------- conv (PE) + bias (ACT) ----------------
        ys_ps = pp.tile([128, ND], F32, tag="ys")
        for h in range(2):
            nc.tensor.matmul(
                out=ys_ps[:, h * 512:(h + 1) * 512],
                lhsT=cw,
                rhs=dp[:, h * 512:(h + 1) * 512],
                start=True, stop=True,
            )
        ys = mid.tile([128, ND], F32, tag="ys_sb")
        nc.scalar.activation(out=ys, in_=ys_ps, func=AF.Identity, bias=cb, scale=1.0)

        # ---------------- interp -> ypad (bf16) ----------------
        Dp = mid.tile([128, ND + 1], F32, tag="Dp")
        nc.vector.memset(Dp[:, 0:1], 0.0)
        nc.vector.memset(Dp[:, ND:ND + 1], 0.0)
        nc.vector.tensor_sub(out=Dp[:, 1:ND], in0=ys[:, 1:ND], in1=ys[:, 0:ND - 1])
        D1 = mid.tile([128, ND + 1], F32, tag="D1")
        D2 = mid.tile([128, ND + 1], F32, tag="D2")
        nc.vector.tensor_scalar_mul(out=D1, in0=Dp, scalar1=0.375)
        nc.vector.tensor_scalar_mul(out=D2, in0=Dp, scalar1=0.125)

        ypad = mid.tile([128, NP], BF, tag="ypad")
        y4 = ypad[:, PAD:PAD + N].rearrange("p (j r) -> p j r", r=4)
        nc.vector.tensor_sub(out=y4[:, :, 0], in0=ys, in1=D1[:, 0:ND])
        nc.vector.tensor_sub(out=y4[:, :, 1], in0=ys, in1=D2[:, 0:ND])
        nc.gpsimd.tensor_add(out=y4[:, :, 2], in0=ys, in1=D2[:, 1:ND + 1])
        nc.gpsimd.tensor_add(out=y4[:, :, 3], in0=ys, in1=D1[:, 1:ND + 1])
        # reflect edges: ypad[2-i] = ypad[4+i], ypad[N+3+i] = ypad[N+1-i]
        for i in range(3):
            nc.vector.tensor_copy(out=ypad[:, 2 - i:3 - i], in_=ypad[:, 4 + i:5 + i])
            nc.vector.tensor_copy(
                out=ypad[:, N + 3 + i:N + 4 + i], in_=ypad[:, N + 1 - i:N + 2 - i])

        # ---------------- stats -> sigmoid scales ----------------
        y_main = ypad[:, PAD:PAD + N]
        sum_y = mid.tile([128, 1], F32, tag="sum_y")
        nc.vector.tensor_reduce(
            out=sum_y, in_=y_main, axis=mybir.AxisListType.X, op=ALU.add)
        dump = pp.tile([128, 2048], F32, tag="pdump")
        sq0 = mid.tile([128, 1], F32, tag="sq0")
        sq1 = mid.tile([128, 1], F32, tag="sq1")
        nc.scalar.activation(out=dump, in_=y_main[:, 0:2048], func=AF.Square,
                             accum_out=sq0)
        dump2 = pp.tile([128, 2048], F32, tag="pdump")
        nc.scalar.activation(out=dump2, in_=y_main[:, 2048:N], func=AF.Square,
                             accum_out=sq1)
        # mean = sum_y/N; var = (sum_y2 - sum_y*mean)/(N-1); f = GAMA/(var+EPS)
        mean = mid.tile([128, 1], F32, tag="mean")
        nc.vector.tensor_scalar_mul(out=mean, in0=sum_y, scalar1=1.0 / N)
        t0 = mid.tile([128, 1], F32, tag="t0")
        nc.vector.tensor_mul(out=t0, in0=sum_y, in1=mean)          # N*mean^2
        t1 = mid.tile([128, 1], F32, tag="t1")
        nc.vector.tensor_add(out=t1, in0=sq0, in1=sq1)             # sum y^2
        t2 = mid.tile([128, 1], F32, tag="t2")
        nc.vector.tensor_sub(out=t2, in0=t1, in1=t0)
        denom = mid.tile([128, 1], F32, tag="denom")
        nc.vector.tensor_scalar(out=denom, in0=t2, scalar1=1.0 / (N - 1),
                                scalar2=EPS, op0=ALU.mult, op1=ALU.add)
        inv = mid.tile([128, 1], F32, tag="inv")
        nc.vector.reciprocal(out=inv, in_=denom)
        f2p = mid.tile([128, 1], F32, tag="f2p")
        f2n = mid.tile([128, 1], F32, tag="f2n")
        nc.vector.tensor_scalar_mul(out=f2p, in0=inv, scalar1=2.0 * GAMA)
        nc.vector.tensor_scalar_mul(out=f2n, in0=inv, scalar1=-2.0 * GAMA)

        # ---------------- gap diffs (bf16) ----------------
        dy1 = loss.tile([128, L3], BF, tag="T1")
        dy2b = loss.tile([128, L3], BF, tag="T2")
        dy3 = loss.tile([128, L3], BF, tag="T3")
        split_tt(nc, "dy", "sub", dy1, ypad[:, 1:1 + L3], ypad[:, 0:L3], L3)
        split_tt(nc, "dy", "sub", dy2b, ypad[:, 3:3 + L3], ypad[:, 1:1 + L3], L3)
        split_tt(nc, "dy", "sub", dy3, ypad[:, 3:3 + L3], ypad[:, 0:L3], L3)

        # ---------------- sigmoids (ACT) + lv (DVE STT) ----------------
        # sa_g = sigmoid(+2f*dy), sb_g = sigmoid(-2f*dy); lv = 4*sa*sb
        # Slot plan: S0 holds sa1 then G32 then nothing big; S1..S5 reused by
        # m-partials and product tiles once the sigmoids are consumed.
        sa1 = loss.tile([128, L3], BF, tag="S0")
        sb1 = loss.tile([128, L3], BF, tag="S1")
        sa2 = loss.tile([128, L3], BF, tag="S2")
        sb2 = loss.tile([128, L3], BF, tag="S3")
        sa3 = loss.tile([128, L3], BF, tag="S4")
        sb3 = loss.tile([128, L3], BF, tag="S5")
        nc.scalar.activation(out=sa1, in_=dy1, func=AF.Sigmoid, scale=f2p)
        nc.scalar.activation(out=sb1, in_=dy1, func=AF.Sigmoid, scale=f2n)
        nc.scalar.activation(out=sa2, in_=dy2b, func=AF.Sigmoid, scale=f2p)
        nc.scalar.activation(out=sb2, in_=dy2b, func=AF.Sigmoid, scale=f2n)
        nc.scalar.activation(out=sa3, in_=dy3, func=AF.Sigmoid, scale=f2p)
        nc.scalar.activation(out=sb3, in_=dy3, func=AF.Sigmoid, scale=f2n)
        lv1 = loss.tile([128, L3], BF, tag="T1")   # reuse dy slots
        lv2b = loss.tile([128, L3], BF, tag="T2")
        lv3 = loss.tile([128, L3], BF, tag="T3")
        nc.vector.scalar_tensor_tensor(
            out=lv1, in0=sa1, scalar=4.0, in1=sb1, op0=ALU.mult, op1=ALU.mult)
        nc.vector.scalar_tensor_tensor(
            out=lv2b, in0=sa2, scalar=4.0, in1=sb2, op0=ALU.mult, op1=ALU.mult)
        nc.vector.scalar_tensor_tensor(
            out=lv3, in0=sa3, scalar=4.0, in1=sb3, op0=ALU.mult, op1=ALU.mult)

        # ---------------- S and G ----------------
        m3 = loss.tile([128, N], BF, tag="S4")      # after sa3/sb3 consumed
        m2 = loss.tile([128, N], BF, tag="S5")
        m1 = loss.tile([128, L3 - 1], BF, tag="M3")
        split_tt(nc, "madd", "add", m3, lv3[:, 0:N], lv3[:, 3:N + 3], N)
        split_tt(nc, "madd", "add", m2, lv2b[:, 0:N], lv2b[:, 2:N + 2], N)
        split_tt(nc, "madd", "add", m1, lv1[:, 0:L3 - 1], lv1[:, 1:L3], L3 - 1)
        s12 = loss.tile([128, N], BF, tag="M4")
        split_tt(nc, "madd", "add", s12, m3, m2, N)
        S32 = mid.tile([128, N], F32, tag="S32")
        nc.vector.scalar_tensor_tensor(
            out=S32, in0=s12, scalar=1.0, in1=m1[:, 2:N + 2],
            op0=ALU.add, op1=ALU.add)
        G32 = loss.tile([128, N], F32, tag="S0")    # after sa1 consumed
        nc.vector.reciprocal_approx_fast(out=G32, in_=S32)
        G = loss.tile([128, N], BF, tag="M4")       # after s12 consumed
        nc.scalar.activation(out=G, in_=G32, func=AF.Copy)

        # ---------------- products + GEMM (two l-halves) ----------------
        H = N // 2
        for half in range(2):
            lo = half * H
            P30 = loss.tile([128, H], BF, tag="S1")
            P21 = loss.tile([128, H], BF, tag="S2")
            P12 = loss.tile([128, H], BF, tag="S3")
            GL1 = loss.tile([128, H], BF, tag="GL1")
            GL2 = loss.tile([128, H], BF, tag="GL2")
            GL3 = loss.tile([128, H], BF, tag="GL3")
            Gh = G[:, lo:lo + H]
            split_tt(nc, "prod", "mul", P30, lv3[:, lo:lo + H], xp[:, lo:lo + H], H)
            split_tt(nc, "prod", "mul", P21, lv2b[:, lo:lo + H], xs1[:, lo:lo + H], H)
            split_tt(nc, "prod", "mul", P12, lv1[:, lo + 2:lo + 2 + H],
                     xp[:, lo + 2:lo + 2 + H], H)
            split_tt(nc, "prod", "mul", GL1, lv1[:, lo + 3:lo + 3 + H], Gh, H)
            split_tt(nc, "prod", "mul", GL2, lv2b[:, lo + 2:lo + 2 + H], Gh, H)
            split_tt(nc, "prod", "mul", GL3, lv3[:, lo + 3:lo + 3 + H], Gh, H)
            W = [wp.tile([128, H], BF, tag=f"W{k}", name=f"W{k}_{half}")
                 for k in range(KS)]
            split_tt(nc, "prod", "mul", W[0], Gh, P30, H)
            split_tt(nc, "prod", "mul", W[1], Gh, P21, H)
            split_tt(nc, "prod", "mul", W[2], Gh, P12, H)
            split_tt(nc, "prod", "mul", W[3], Gh, xs1[:, lo + 2:lo + 2 + H], H)
            split_tt(nc, "prod", "mul", W[4], GL1, xp[:, lo + 4:lo + 4 + H], H)
            split_tt(nc, "prod", "mul", W[5], GL2, xs1[:, lo + 4:lo + 4 + H], H)
            split_tt(nc, "prod", "mul", W[6], GL3, xp[:, lo + 6:lo + 6 + H], H)

            for b in range(2):
                prow = slice(64 * b, 64 * (b + 1))
                for chn in range(H // 512):
                    acc = ppa.tile([128, 512], F32, tag="acc")
                    cs = slice(chn * 512, (chn + 1) * 512)
                    for k in range(KS):
                        nc.tensor.matmul(
                            out=acc,
                            lhsT=fck[prow, k, :],
                            rhs=W[k][prow, cs],
                            start=(k == 0), stop=(k == KS - 1),
                        )
                    stage = stp.tile([128, 512], F32, tag="stage",
                                    name=f"stage_{half}_{b}_{chn}")
                    if (b + chn) % 2 == 0:
                        nc.scalar.copy(out=stage, in_=acc)
                    else:
                        nc.vector.tensor_copy(out=stage, in_=acc)
                    nc.sync.dma_start(
                        out=out_d[:, b, lo + chn * 512:lo + (chn + 1) * 512],
                        in_=stage)


def build_nc():
    nc = bacc_mod.Bacc(None, target_bir_lowering=False)
    xp_d = nc.dram_tensor("xp", [128, NP], BF, kind="ExternalInput")
    cwdp_d = nc.dram_tensor("cwdp", [32, 128 + ND], F32, kind="ExternalInput")
    cb_d = nc.dram_tensor("cb", [128, 1], F32, kind="ExternalInput")
    fck_d = nc.dram_tensor("fck", [128, KS, 128], BF, kind="ExternalInput")
    out_d = nc.dram_tensor("out", [128, 2, N], F32, kind="ExternalOutput")
    with tile.TileContext(nc) as tc:
        kernel_body(tc, xp_d, cwdp_d, cb_d, fck_d, out_d)
    nc.compile()
    return nc


def prep_inputs(deep, x, conv_w, conv_b, fc_w):
    deep = np.asarray(deep, np.float32)
    x = np.asarray(x, np.float32)
    conv_w = np.asarray(conv_w, np.float32)
    conv_b = np.asarray(conv_b, np.float32)
    fc_w = np.asarray(fc_w, np.float32)

    xpad = np.pad(x, ((0, 0), (0, 0), (PAD, PAD)), mode="reflect")
    xp_all = np.ascontiguousarray(xpad.reshape(NCORES, 128, NP)).astype(bf16)
    dp_all = np.ascontiguousarray(deep.reshape(NCORES, 32, ND))
    cw_blk = np.zeros((32, 128), np.float32)
    cw_blk[0:16, 0:64] = conv_w.T
    cw_blk[16:32, 64:128] = conv_w.T
    cb = np.ascontiguousarray(
        np.concatenate([conv_b, conv_b]).reshape(128, 1).astype(np.float32))
    fc3 = fc_w.reshape(128, 64, KS)
    fck_half = np.transpose(fc3, (1, 2, 0))
    fck = np.ascontiguousarray(
        np.concatenate([fck_half, fck_half], axis=0)).astype(bf16)
    return [
        {"xp": np.ascontiguousarray(xp_all[ci]),
         "cwdp": np.ascontiguousarray(
             np.concatenate([cw_blk, dp_all[ci]], axis=1)),
         "cb": cb, "fck": fck}
        for ci in range(NCORES)
    ]


def gather_out(results):
    out_full = np.empty((16, 128, N), np.float32)
    for ci in range(NCORES):
        o = results[ci]["out"]
        out_full[2 * ci] = o[:, 0]
        out_full[2 * ci + 1] = o[:, 1]
    return out_full


_CACHED = {}


def _get_nc():
    if "nc" not in _CACHED:
        _CACHED["nc"] = build_nc()
    return _CACHED["nc"]


def kernel(deep, x, conv_w, conv_b, fc_w):
    in_maps = prep_inputs(deep, x, conv_w, conv_b, fc_w)
    nc = _get_nc()
    res = run_bass_kernel_spmd(nc, in_maps, core_ids=list(range(NCORES)))
    return gather_out(res.results)
